# revision 1
# baseline (speedup 1.0000x reference)
"""TRN2 Bass kernel for nn_BSLinear_71159018160311.

Computes  out = input @ W.T  with
  W = U @ diag(weight^2 * mask) @ Vh + U_additional @ Vh_additional

Sharding: data-parallel over the B*S=16384 token dim across 8 NeuronCores
(2048 tokens/core), no collectives. Each core runs the factorized form as
two fused matmul phases in float32r (full-rate fp32 streaming on the PE):

  phase 1: t = V_eff @ x_c.T   kept entirely in SBUF (r-major, [RP, 2048])
           k-blocked PSUM accumulation (4 k-tiles/block) + SBUF adds
  phase 2: yT_c = U_eff @ t    (ut streamed once, 512-col chunks; output
           dout-major, host transposes back)

Both phases share one PSUM pool (same tag), so phase-2 matmuls start in the
PSUM buffer freed mid-way through phase 1's last block instead of stalling
on a pool-boundary WAR against the final accumulate-adds.

V_eff = [Vh; Vh_additional(pad)]  (rows), U_eff = [U*s, U_additional(pad)]
(cols), s = weight^2*mask folded on host. When U_additional/Vh_additional
are all-zero (they are for this problem instance), the padded tail is
dropped (NR=8 -> RP=1024), saving 11% of the matmul work; otherwise the
NR=9 (RP=1152) program handles the full module.

HBM traffic per core is at the floor: x 32MB + vt 16MB + ut 16MB + y 32MB
(the [RP,2048] intermediate never touches DRAM).
"""

import functools

import numpy as np

B, S, D_IN, D_OUT, R, A = 4, 4096, 4096, 4096, 1024, 64
N_CORES = 8
T = B * S
TC = T // N_CORES  # 2048
KT = D_IN // 128  # 32
KB = 4
NB = KT // KB
NN = TC // 512  # 4
ND = D_OUT // 512  # 8


@functools.lru_cache(maxsize=2)
def _build(NR):
    import concourse.bacc as bacc
    import concourse.mybir as mybir
    import concourse.tile as tile

    RP = NR * 128
    f32r = mybir.dt.float32r
    f32 = mybir.dt.float32
    add = mybir.AluOpType.add

    nc = bacc.Bacc(trn_type="TRN2")
    with tile.TileContext(nc) as tc:
        with tc.tile_pool(name="dram", bufs=1, space="DRAM") as dram:
            xT = dram.tile([D_IN, TC], f32r, kind="ExternalInput", name="xT")
            vt = dram.tile([D_IN, RP], f32r, kind="ExternalInput", name="vt")
            ut = dram.tile([RP, D_OUT], f32r, kind="ExternalInput", name="ut")
            yT = dram.tile([D_OUT, TC], f32, kind="ExternalOutput", name="yT")

            with (
                tc.tile_pool(name="tsb", bufs=NR) as tpool,
                tc.tile_pool(name="ut0", bufs=1) as u0pool,
                tc.tile_pool(name="ps", bufs=2, space="PSUM") as pspool,
            ):
                t_sb = [tpool.tile([128, TC], f32r, name="tsb") for _ in range(NR)]
                # first ut chunk: loads during phase 1 (own address space);
                # DMA emitted after block-0 loads so it doesn't delay startup
                ut0 = u0pool.tile([128, NR, 512], f32r)

                # ---- phase 1 ----
                with (
                    tc.tile_pool(name="xk", bufs=2 * KB) as xpool,
                    tc.tile_pool(name="vk", bufs=2 * KB) as vpool,
                ):
                    for kb in range(NB):
                        xts, vts = [], []
                        for j in range(KB):
                            k = kb * KB + j
                            xt_t = xpool.tile([128, TC], f32r, name="xk")
                            nc.sync.dma_start(xt_t[:], xT[k * 128:(k + 1) * 128, :])
                            vt_t = vpool.tile([128, RP], f32r, name="vk")
                            nc.sync.dma_start(vt_t[:], vt[k * 128:(k + 1) * 128, :])
                            xts.append(xt_t)
                            vts.append(vt_t)
                        if kb == 0:
                            nc.sync.dma_start(
                                ut0[:],
                                ut[:, 0:512].rearrange("(ko p) f -> p ko f", p=128),
                            )
                        for r in range(NR):
                            psum = pspool.tile([128, NN, 512], f32, name="ps")
                            for j in range(KB):
                                for n in range(NN):
                                    nc.tensor.matmul(
                                        psum[:, n, :],
                                        lhsT=vts[j][:, r * 128:(r + 1) * 128],
                                        rhs=xts[j][:, n * 512:(n + 1) * 512],
                                        start=(j == 0),
                                        stop=(j == KB - 1),
                                    )
                            dst = t_sb[r][:, :]
                            pflat = psum.rearrange("p a b -> p (a b)")
                            if kb == 0:
                                nc.any.tensor_copy(dst, pflat)
                            else:
                                nc.any.tensor_tensor(dst, dst, pflat, add)

                # ---- phase 2 (ut stationary, t moving; output dout-major) ----
                with (
                    tc.tile_pool(name="utd", bufs=2) as upool,
                    tc.tile_pool(name="ysb", bufs=8) as ypool,
                ):
                    for d in range(ND):
                        if d == 0:
                            ut_t = ut0
                        else:
                            ut_t = upool.tile([128, NR, 512], f32r, name="utd")
                            nc.sync.dma_start(
                                ut_t[:],
                                ut[:, d * 512:(d + 1) * 512].rearrange(
                                    "(ko p) f -> p ko f", p=128
                                ),
                            )
                        for dd in range(4):  # 128-wide dout sub-blocks
                            psum = pspool.tile([128, NN, 512], f32, name="ps")
                            for r in range(NR):
                                for n in range(NN):
                                    nc.tensor.matmul(
                                        psum[:, n, :],
                                        lhsT=ut_t[:, r, dd * 128:(dd + 1) * 128],
                                        rhs=t_sb[r][:, n * 512:(n + 1) * 512],
                                        start=(r == 0),
                                        stop=(r == NR - 1),
                                    )
                            row = d * 512 + dd * 128
                            for n in range(NN):
                                ysb = ypool.tile([128, 512], f32, name="ysb")
                                nc.any.tensor_copy(ysb[:], psum[:, n, :])
                                nc.sync.dma_start(
                                    yT[row : row + 128, n * 512:(n + 1) * 512],
                                    ysb[:],
                                )
    nc.compile()
    return nc, xT.name, vt.name, ut.name, yT.name


def _prep_maps(input, weight, U, Vh, U_additional, Vh_additional, mask, names, NR):
    xT_name, vt_name, ut_name = names
    RP = NR * 128
    s = weight * weight * mask
    U_eff = np.zeros((D_OUT, RP), np.float32)
    U_eff[:, :R] = U * s[None, :]
    V_eff = np.zeros((RP, D_IN), np.float32)
    V_eff[:R] = Vh
    if NR > R // 128:
        U_eff[:, R : R + A] = U_additional
        V_eff[R : R + A] = Vh_additional
    vt = np.ascontiguousarray(V_eff.T)
    ut = np.ascontiguousarray(U_eff.T)
    x2 = np.asarray(input, dtype=np.float32).reshape(T, D_IN)
    in_maps = []
    for c in range(N_CORES):
        xTc = np.ascontiguousarray(x2[c * TC : (c + 1) * TC].T)
        in_maps.append({xT_name: xTc, vt_name: vt, ut_name: ut})
    return in_maps


def _gather(results, yT_name):
    out = np.empty((T, D_OUT), np.float32)
    for c in range(N_CORES):
        out[c * TC : (c + 1) * TC] = results[c][yT_name].T
    return out.reshape(B, S, D_OUT)


def _pick_nr(U_additional, Vh_additional):
    if not np.asarray(U_additional).any() or not np.asarray(Vh_additional).any():
        return R // 128  # additional term contributes nothing
    return (R + A + 127) // 128


def kernel(input, weight, U, Vh, U_additional, Vh_additional, mask, **_kw):
    from concourse.bass_utils import run_bass_kernel_spmd

    input = np.asarray(input, dtype=np.float32)
    weight = np.asarray(weight, dtype=np.float32)
    U = np.asarray(U, dtype=np.float32)
    Vh = np.asarray(Vh, dtype=np.float32)
    U_additional = np.asarray(U_additional, dtype=np.float32)
    Vh_additional = np.asarray(Vh_additional, dtype=np.float32)
    mask = np.asarray(mask, dtype=np.float32)

    NR = _pick_nr(U_additional, Vh_additional)
    nc, xT_name, vt_name, ut_name, yT_name = _build(NR)
    in_maps = _prep_maps(
        input, weight, U, Vh, U_additional, Vh_additional, mask,
        (xT_name, vt_name, ut_name), NR,
    )
    res = run_bass_kernel_spmd(nc, in_maps, core_ids=list(range(N_CORES)))
    return _gather(res.results, yT_name)



# revision 2
# speedup vs baseline: 1.3921x; 1.3921x over previous
"""Hybrid precision TRN2 kernel for nn_BSLinear_71159018160311.

out = input @ W.T,  W = U diag(weight^2*mask) Vh + U_add Vh_add.

Rank-1024 factorized form, data-parallel over tokens (2048/core, 8 cores).
Components r are ranked by energy c_r = s_r*|U_r|*|V_r|; the top R32=512
("main") run in bf16 at full PE rate, the bottom R8=512 ("tail") run in
bare e4m3 fp8 with DoubleRow matmuls (2 k-subtiles per instruction, ~2x
MAC rate). The tail carries ~4.7% of W's energy, so its ~5% fp8
quantization noise contributes only ~1.2% end-to-end error (gate: 2e-2).

Scale plumbing (all powers of two, exact):
  x8 = Q(x*sx), v8 = Q(V_tail*sv), u8 = Q(U_tail*su)
  phase-1 tail PSUM = t_tail*sx*sv -> t8 = Q(psum*alpha), alpha = st/(sx*sv)
  main U is pre-multiplied by st*su so main and tail share one PSUM scale;
  the phase-2 drain multiplies by 1/(st*su).
"""

import functools

import numpy as np
import ml_dtypes

B, S, D_IN, D_OUT, R, A = 4, 4096, 4096, 4096, 1024, 64
N_CORES = 8
T = B * S
TC = T // N_CORES  # 2048
KT = D_IN // 128  # 32 k-tiles
DKT = KT // 2  # 16 double-row k-tiles
MT = 4  # main r-tiles (R32 = 512)
TT = 4  # tail r-tiles (R8 = 512)
NRHO = TT // 2  # tail double-row r-tiles
NB_H = 4  # phase-1 blocks
KB_H = KT // NB_H  # 8 k-tiles per block
DKB_H = DKT // NB_H  # 4 dr k-tiles per block
NN = TC // 512  # 4
ND = D_OUT // 512  # 8

F8 = ml_dtypes.float8_e4m3fn
BF16 = ml_dtypes.bfloat16


@functools.lru_cache(maxsize=8)
def _build_hybrid(alpha, inv, reps=1):
    import concourse.bacc as bacc
    import concourse.mybir as mybir
    import concourse.tile as tile

    f32 = mybir.dt.float32
    bf = mybir.dt.bfloat16
    f8 = mybir.dt.float8e4
    add = mybir.AluOpType.add
    mult = mybir.AluOpType.mult
    DR = mybir.MatmulPerfMode.DoubleRow

    nc = bacc.Bacc(trn_type="TRN2")
    with tile.TileContext(nc) as tc:
        with tc.tile_pool(name="dram", bufs=1, space="DRAM") as dram:
            xb = dram.tile([D_IN, TC], bf, kind="ExternalInput", name="xb")
            x8 = dram.tile([DKT * 128, 2, TC], f8, kind="ExternalInput", name="x8")
            vb = dram.tile([D_IN, MT * 128], bf, kind="ExternalInput", name="vb")
            v8 = dram.tile([DKT * 128, 2, TT * 128], f8, kind="ExternalInput", name="v8")
            ub = dram.tile([128, MT, D_OUT], bf, kind="ExternalInput", name="ub")
            u8 = dram.tile([128, NRHO, 2, D_OUT], f8, kind="ExternalInput", name="u8")
            yT = dram.tile([D_OUT, TC], f32, kind="ExternalOutput", name="yT")

            with (
                tc.tile_pool(name="tm32", bufs=1) as tm32p,
                tc.tile_pool(name="tt32", bufs=1) as tt32p,
                tc.tile_pool(name="tbm", bufs=1) as tbmp,
                tc.tile_pool(name="t8", bufs=NRHO) as t8p,
                tc.tile_pool(name="u0a", bufs=1) as u0ap,
                tc.tile_pool(name="u0b", bufs=1) as u0bp,
                tc.tile_pool(name="ps", bufs=2, space="PSUM") as pspool,
            ):
                t32m = tm32p.tile([128, MT, TC], f32)
                t32t = tt32p.tile([128, TT, TC], f32)
                tbm = tbmp.tile([128, MT, TC], bf)
                t8s = [t8p.tile([128, 2, TC], f8, name="t8") for _ in range(NRHO)]

                with (
                    tc.tile_pool(name="xbk", bufs=KB_H + 2) as xbp,
                    tc.tile_pool(name="x8k", bufs=DKB_H + 2) as x8p,
                    tc.tile_pool(name="vbk", bufs=KB_H + 2) as vbp,
                    tc.tile_pool(name="v8k", bufs=DKB_H + 2) as v8p,
                    tc.tile_pool(name="ubd", bufs=2) as ubp,
                    tc.tile_pool(name="u8d", bufs=2) as u8p2,
                    tc.tile_pool(name="ysb", bufs=8) as ypool,
                ):
                  for rep in range(reps):
                    ub0 = u0ap.tile([128, MT, 512], bf)
                    u80 = u0bp.tile([128, NRHO, 2, 512], f8)
                    # ---- phase 1 ----
                    for kb in range(NB_H):
                        xts, vts, x8ts, v8ts = [], [], [], []
                        for j in range(KB_H):
                            k = kb * KB_H + j
                            xt = xbp.tile([128, TC], bf, name="xbk")
                            nc.sync.dma_start(xt[:], xb[k * 128:(k + 1) * 128, :])
                            vt = vbp.tile([128, MT * 128], bf, name="vbk")
                            nc.sync.dma_start(vt[:], vb[k * 128:(k + 1) * 128, :])
                            xts.append(xt)
                            vts.append(vt)
                        for j in range(DKB_H):
                            dk = kb * DKB_H + j
                            x8t = x8p.tile([128, 2, TC], f8, name="x8k")
                            nc.sync.dma_start(
                                x8t[:], x8[dk * 128:(dk + 1) * 128, :, :]
                            )
                            v8t = v8p.tile([128, 2, TT * 128], f8, name="v8k")
                            nc.sync.dma_start(
                                v8t[:], v8[dk * 128:(dk + 1) * 128, :, :]
                            )
                            x8ts.append(x8t)
                            v8ts.append(v8t)
                        if kb == 0:
                            nc.sync.dma_start(ub0[:], ub[:, :, 0:512])
                            nc.sync.dma_start(u80[:], u8[:, :, :, 0:512])

                        # main r-tiles (bf16)
                        for rt in range(MT):
                            psum = pspool.tile([128, NN, 512], f32, name="ps")
                            for j in range(KB_H):
                                for n in range(NN):
                                    nc.tensor.matmul(
                                        psum[:, n, :],
                                        lhsT=vts[j][:, rt * 128:(rt + 1) * 128],
                                        rhs=xts[j][:, n * 512:(n + 1) * 512],
                                        start=(j == 0),
                                        stop=(j == KB_H - 1),
                                    )
                            dst = t32m[:, rt, :]
                            pflat = psum.rearrange("p a b -> p (a b)")
                            if kb == 0:
                                nc.vector.tensor_copy(dst, pflat)
                            elif kb < NB_H - 1:
                                nc.vector.tensor_tensor(dst, dst, pflat, add)
                            else:
                                nc.vector.tensor_tensor(
                                    tbm[:, rt, :], dst, pflat, add
                                )
                        # tail r-tiles (fp8 DoubleRow)
                        for rt in range(TT):
                            psum = pspool.tile([128, NN, 512], f32, name="ps")
                            for j in range(DKB_H):
                                for n in range(NN):
                                    nc.tensor.matmul(
                                        psum[:, n, :],
                                        lhsT=v8ts[j][:, :, rt * 128:(rt + 1) * 128],
                                        rhs=x8ts[j][:, :, n * 512:(n + 1) * 512],
                                        start=(j == 0),
                                        stop=(j == DKB_H - 1),
                                        perf_mode=DR,
                                    )
                            dst = t32t[:, rt, :]
                            pflat = psum.rearrange("p a b -> p (a b)")
                            if kb == 0:
                                nc.vector.tensor_copy(dst, pflat)
                            else:
                                nc.vector.tensor_tensor(dst, dst, pflat, add)
                            if kb == NB_H - 1:
                                nc.scalar.activation(
                                    t8s[rt // 2][:, rt % 2, :],
                                    dst,
                                    mybir.ActivationFunctionType.Copy,
                                    scale=float(alpha),
                                )

                    # ---- phase 2 ----
                    for d in range(ND):
                        if d == 0:
                            ub_t, u8_t = ub0, u80
                        else:
                            ub_t = ubp.tile([128, MT, 512], bf, name="ubd")
                            nc.sync.dma_start(
                                ub_t[:], ub[:, :, d * 512:(d + 1) * 512]
                            )
                            u8_t = u8p2.tile([128, NRHO, 2, 512], f8, name="u8d")
                            nc.sync.dma_start(
                                u8_t[:], u8[:, :, :, d * 512:(d + 1) * 512]
                            )
                        for dd in range(4):
                            psum = pspool.tile([128, NN, 512], f32, name="ps")
                            for rt in range(MT):
                                for n in range(NN):
                                    nc.tensor.matmul(
                                        psum[:, n, :],
                                        lhsT=ub_t[:, rt, dd * 128:(dd + 1) * 128],
                                        rhs=tbm[:, rt, n * 512:(n + 1) * 512],
                                        start=(rt == 0),
                                        stop=False,
                                    )
                            for rho in range(NRHO):
                                for n in range(NN):
                                    nc.tensor.matmul(
                                        psum[:, n, :],
                                        lhsT=u8_t[:, rho, :, dd * 128:(dd + 1) * 128],
                                        rhs=t8s[rho][:, :, n * 512:(n + 1) * 512],
                                        start=False,
                                        stop=(rho == NRHO - 1),
                                        perf_mode=DR,
                                    )
                            row = d * 512 + dd * 128
                            for n in range(NN):
                                ysb = ypool.tile([128, 512], f32, name="ysb")
                                nc.vector.tensor_scalar(
                                    ysb[:], psum[:, n, :], float(inv), None, mult
                                )
                                nc.sync.dma_start(
                                    yT[row : row + 128, n * 512:(n + 1) * 512],
                                    ysb[:],
                                )
    nc.compile()
    names = dict(
        xb=xb.name, x8=x8.name, vb=vb.name, v8=v8.name, ub=ub.name,
        u8=u8.name, yT=yT.name,
    )
    return nc, names


def _pow2(maxabs, target=224.0):
    return float(2.0 ** np.floor(np.log2(target / max(float(maxabs), 1e-30))))


def _q8(a, scale):
    return np.clip(a * scale, -240.0, 240.0).astype(F8)


def _select(weight, mask, U, Vh, U_additional, Vh_additional):
    """Rank components by energy; returns (main_idx, tail_idx, est_err) over
    the combined 1024+64 component set, or None if hybrid doesn't apply."""
    s = (weight.astype(np.float64) ** 2) * mask.astype(np.float64)
    cU = np.linalg.norm(U.astype(np.float64), axis=0)
    cV = np.linalg.norm(Vh.astype(np.float64), axis=1)
    c = s * cU * cV
    caU = np.linalg.norm(U_additional.astype(np.float64), axis=0)
    caV = np.linalg.norm(Vh_additional.astype(np.float64), axis=1)
    ca = caU * caV
    call = np.concatenate([c, ca])
    order = np.argsort(-call)
    e = call[order] ** 2
    etot = e.sum()
    if etot <= 0:
        return None
    n_main, n_tail = MT * 128, TT * 128
    if len(order) < n_main + n_tail:
        return None
    main = order[:n_main]
    tail = order[n_main : n_main + n_tail]
    drop_e = e[n_main + n_tail :].sum() / etot
    tail_e = e[n_main : n_main + n_tail].sum() / etot
    # coeffs: bare-fp8 tail noise (~5.3%)^2 of tail energy; bf16 main ~0.3%
    est = np.sqrt(drop_e + 0.0031 * tail_e + 0.003**2)
    return main, tail, est


def _scales(input, weight, U, Vh, U_additional, Vh_additional, mask, main, tail):
    s = weight * weight * mask
    Ucat = np.concatenate([U * s[None, :], U_additional], axis=1)
    Vcat = np.concatenate([Vh, Vh_additional], axis=0)
    Ut = Ucat[:, tail]
    Vt = Vcat[tail]
    x2 = np.asarray(input, np.float32).reshape(T, D_IN)
    sx = _pow2(np.abs(x2).max())
    sv = _pow2(np.abs(Vt).max())
    su = _pow2(np.abs(Ut).max())
    xnorm = np.linalg.norm(x2, axis=1).max()
    vnorm = np.linalg.norm(Vt, axis=1).max()
    st = _pow2(xnorm * vnorm)
    return sx, sv, su, st


def _prep_hybrid(input, weight, U, Vh, U_additional, Vh_additional, mask,
                 main, tail, names):
    s = weight * weight * mask
    Ucat = np.concatenate([U * s[None, :], U_additional], axis=1)
    Vcat = np.concatenate([Vh, Vh_additional], axis=0)
    Um, Ut = Ucat[:, main], Ucat[:, tail]  # [D_OUT, 512]
    Vm, Vt = Vcat[main], Vcat[tail]  # [512, D_IN]

    x2 = np.asarray(input, np.float32).reshape(T, D_IN)
    sx = _pow2(np.abs(x2).max())
    sv = _pow2(np.abs(Vt).max())
    su = _pow2(np.abs(Ut).max())
    xnorm = np.linalg.norm(x2, axis=1).max()
    vnorm = np.linalg.norm(Vt, axis=1).max()
    st = _pow2(xnorm * vnorm)
    alpha = st / (sx * sv)
    stsu = st * su

    # weights (replicated across cores)
    vb = np.ascontiguousarray(Vm.T.astype(BF16))  # [D_IN, 512]
    v8 = np.ascontiguousarray(
        _q8(Vt.T, sv).reshape(DKT, 2, 128, TT * 128).transpose(0, 2, 1, 3)
        .reshape(DKT * 128, 2, TT * 128)
    )
    ub = np.ascontiguousarray(
        (Um.T * stsu).astype(BF16).reshape(MT, 128, D_OUT).transpose(1, 0, 2)
    )  # [128, MT, D_OUT]
    u8 = np.ascontiguousarray(
        _q8(Ut.T, su).reshape(NRHO, 2, 128, D_OUT).transpose(2, 0, 1, 3)
    )  # [128, NRHO, 2, D_OUT]
    in_maps = []
    for c in range(N_CORES):
        xcT = np.ascontiguousarray(x2[c * TC:(c + 1) * TC].T)  # [D_IN, TC]
        xb = xcT.astype(BF16)
        x8 = np.ascontiguousarray(
            _q8(xcT, sx).reshape(DKT, 2, 128, TC).transpose(0, 2, 1, 3)
            .reshape(DKT * 128, 2, TC)
        )
        in_maps.append({
            names["xb"]: xb, names["x8"]: x8, names["vb"]: vb,
            names["v8"]: v8, names["ub"]: ub, names["u8"]: u8,
        })
    return in_maps, alpha, 1.0 / stsu


def _gather(results, yT_name):
    out = np.empty((T, D_OUT), np.float32)
    for c in range(N_CORES):
        out[c * TC:(c + 1) * TC] = results[c][yT_name].T
    return out.reshape(B, S, D_OUT)


def _kernel_hybrid(input, weight, U, Vh, U_additional, Vh_additional, mask):
    from concourse.bass_utils import run_bass_kernel_spmd

    input = np.asarray(input, np.float32)
    weight = np.asarray(weight, np.float32)
    U = np.asarray(U, np.float32)
    Vh = np.asarray(Vh, np.float32)
    U_additional = np.asarray(U_additional, np.float32)
    Vh_additional = np.asarray(Vh_additional, np.float32)
    mask = np.asarray(mask, np.float32)

    sel = _select(weight, mask, U, Vh, U_additional, Vh_additional)
    main, tail, est = sel
    sx, sv, su, st = _scales(
        input, weight, U, Vh, U_additional, Vh_additional, mask, main, tail
    )
    alpha, inv = st / (sx * sv), 1.0 / (st * su)
    nc, names = _build_hybrid(alpha, inv)
    in_maps, a2, i2 = _prep_hybrid(
        input, weight, U, Vh, U_additional, Vh_additional, mask,
        main, tail, names,
    )
    assert a2 == alpha and i2 == inv
    res = run_bass_kernel_spmd(nc, in_maps, core_ids=list(range(N_CORES)))
    return _gather(res.results, names["yT"])



# baseline (fp32r) fallback constants — distinct from the hybrid block sizes
KB = 4
NB = KT // KB  # 8

@functools.lru_cache(maxsize=2)
def _build_base(NR):
    import concourse.bacc as bacc
    import concourse.mybir as mybir
    import concourse.tile as tile

    RP = NR * 128
    f32r = mybir.dt.float32r
    f32 = mybir.dt.float32
    add = mybir.AluOpType.add

    nc = bacc.Bacc(trn_type="TRN2")
    with tile.TileContext(nc) as tc:
        with tc.tile_pool(name="dram", bufs=1, space="DRAM") as dram:
            xT = dram.tile([D_IN, TC], f32r, kind="ExternalInput", name="xT")
            vt = dram.tile([D_IN, RP], f32r, kind="ExternalInput", name="vt")
            ut = dram.tile([RP, D_OUT], f32r, kind="ExternalInput", name="ut")
            yT = dram.tile([D_OUT, TC], f32, kind="ExternalOutput", name="yT")

            with (
                tc.tile_pool(name="tsb", bufs=NR) as tpool,
                tc.tile_pool(name="ut0", bufs=1) as u0pool,
                tc.tile_pool(name="ps", bufs=2, space="PSUM") as pspool,
            ):
                t_sb = [tpool.tile([128, TC], f32r, name="tsb") for _ in range(NR)]
                # first ut chunk: loads during phase 1 (own address space);
                # DMA emitted after block-0 loads so it doesn't delay startup
                ut0 = u0pool.tile([128, NR, 512], f32r)

                # ---- phase 1 ----
                with (
                    tc.tile_pool(name="xk", bufs=2 * KB) as xpool,
                    tc.tile_pool(name="vk", bufs=2 * KB) as vpool,
                ):
                    for kb in range(NB):
                        xts, vts = [], []
                        for j in range(KB):
                            k = kb * KB + j
                            xt_t = xpool.tile([128, TC], f32r, name="xk")
                            nc.sync.dma_start(xt_t[:], xT[k * 128:(k + 1) * 128, :])
                            vt_t = vpool.tile([128, RP], f32r, name="vk")
                            nc.sync.dma_start(vt_t[:], vt[k * 128:(k + 1) * 128, :])
                            xts.append(xt_t)
                            vts.append(vt_t)
                        if kb == 0:
                            nc.sync.dma_start(
                                ut0[:],
                                ut[:, 0:512].rearrange("(ko p) f -> p ko f", p=128),
                            )
                        for r in range(NR):
                            psum = pspool.tile([128, NN, 512], f32, name="ps")
                            for j in range(KB):
                                for n in range(NN):
                                    nc.tensor.matmul(
                                        psum[:, n, :],
                                        lhsT=vts[j][:, r * 128:(r + 1) * 128],
                                        rhs=xts[j][:, n * 512:(n + 1) * 512],
                                        start=(j == 0),
                                        stop=(j == KB - 1),
                                    )
                            dst = t_sb[r][:, :]
                            pflat = psum.rearrange("p a b -> p (a b)")
                            if kb == 0:
                                nc.any.tensor_copy(dst, pflat)
                            else:
                                nc.any.tensor_tensor(dst, dst, pflat, add)

                # ---- phase 2 (ut stationary, t moving; output dout-major) ----
                with (
                    tc.tile_pool(name="utd", bufs=2) as upool,
                    tc.tile_pool(name="ysb", bufs=8) as ypool,
                ):
                    for d in range(ND):
                        if d == 0:
                            ut_t = ut0
                        else:
                            ut_t = upool.tile([128, NR, 512], f32r, name="utd")
                            nc.sync.dma_start(
                                ut_t[:],
                                ut[:, d * 512:(d + 1) * 512].rearrange(
                                    "(ko p) f -> p ko f", p=128
                                ),
                            )
                        for dd in range(4):  # 128-wide dout sub-blocks
                            psum = pspool.tile([128, NN, 512], f32, name="ps")
                            for r in range(NR):
                                for n in range(NN):
                                    nc.tensor.matmul(
                                        psum[:, n, :],
                                        lhsT=ut_t[:, r, dd * 128:(dd + 1) * 128],
                                        rhs=t_sb[r][:, n * 512:(n + 1) * 512],
                                        start=(r == 0),
                                        stop=(r == NR - 1),
                                    )
                            row = d * 512 + dd * 128
                            for n in range(NN):
                                ysb = ypool.tile([128, 512], f32, name="ysb")
                                nc.any.tensor_copy(ysb[:], psum[:, n, :])
                                nc.sync.dma_start(
                                    yT[row : row + 128, n * 512:(n + 1) * 512],
                                    ysb[:],
                                )
    nc.compile()
    return nc, xT.name, vt.name, ut.name, yT.name


def _prep_maps_base(input, weight, U, Vh, U_additional, Vh_additional, mask, names, NR):
    xT_name, vt_name, ut_name = names
    RP = NR * 128
    s = weight * weight * mask
    U_eff = np.zeros((D_OUT, RP), np.float32)
    U_eff[:, :R] = U * s[None, :]
    V_eff = np.zeros((RP, D_IN), np.float32)
    V_eff[:R] = Vh
    if NR > R // 128:
        U_eff[:, R : R + A] = U_additional
        V_eff[R : R + A] = Vh_additional
    vt = np.ascontiguousarray(V_eff.T)
    ut = np.ascontiguousarray(U_eff.T)
    x2 = np.asarray(input, dtype=np.float32).reshape(T, D_IN)
    in_maps = []
    for c in range(N_CORES):
        xTc = np.ascontiguousarray(x2[c * TC : (c + 1) * TC].T)
        in_maps.append({xT_name: xTc, vt_name: vt, ut_name: ut})
    return in_maps


def _gather_base(results, yT_name):
    out = np.empty((T, D_OUT), np.float32)
    for c in range(N_CORES):
        out[c * TC : (c + 1) * TC] = results[c][yT_name].T
    return out.reshape(B, S, D_OUT)


def _pick_nr_base(U_additional, Vh_additional):
    if not np.asarray(U_additional).any() or not np.asarray(Vh_additional).any():
        return R // 128  # additional term contributes nothing
    return (R + A + 127) // 128




def kernel(input, weight, U, Vh, U_additional, Vh_additional, mask, **_kw):
    """Full-input entry point: hybrid bf16+fp8 path when the component
    energy split supports it within the error budget, else exact fp32r."""
    from concourse.bass_utils import run_bass_kernel_spmd

    input = np.asarray(input, np.float32)
    weight = np.asarray(weight, np.float32)
    U = np.asarray(U, np.float32)
    Vh = np.asarray(Vh, np.float32)
    U_additional = np.asarray(U_additional, np.float32)
    Vh_additional = np.asarray(Vh_additional, np.float32)
    mask = np.asarray(mask, np.float32)

    sel = None
    if (input.shape, U.shape, Vh.shape) == ((B, S, D_IN), (D_OUT, R), (R, D_IN)):
        sel = _select(weight, mask, U, Vh, U_additional, Vh_additional)
    if sel is not None and sel[2] < 1.5e-2:
        return _kernel_hybrid(
            input, weight, U, Vh, U_additional, Vh_additional, mask
        )

    NR = _pick_nr_base(U_additional, Vh_additional)
    nc, xT_name, vt_name, ut_name, yT_name = _build_base(NR)
    in_maps = _prep_maps_base(
        input, weight, U, Vh, U_additional, Vh_additional, mask,
        (xT_name, vt_name, ut_name), NR,
    )
    res = run_bass_kernel_spmd(nc, in_maps, core_ids=list(range(N_CORES)))
    return _gather_base(res.results, yT_name)
